# revision 57
# baseline (speedup 1.0000x reference)
"""Multi-head attention (B=2, S=2048, D=1024, H=16) on 8 Trainium2 cores.

Sharding: head x batch. Core c handles heads {2c, 2c+1} for BOTH batches
(instead of 4 heads x 1 batch). This makes the post-attention exchange a
clean 8-way AllToAll per batch with zero redundancy: core c sends its
[128ch, 512q] blocks and receives exactly its [1024ch, 256q] output slice
(core c owns queries [256c, 256c+256) of each batch). Compared to the
batch-split sharding this halves the A2A payload (2 x 512KB, batch-0's
exchange hidden under batch-1 attention) and halves the output projection
(contract 1024 real channels, no zero-padded half).

Per core:
  1. Projections, interleaved with attention so the PE never idles:
     qw^T/kw^T in transposed layout (bf16 -- full-rate scores matmuls and
     half-size LDWEIGHTS vs fp32r), bias fused via DVE per-partition add;
     vw first as vw^T (N=512 matmuls, per-partition DVE bias), then PE
     transposes [128,128] blocks into the natural [kpos, ch] layout the
     PV matmul needs, with static memset ones columns for the fused
     softmax-denominator sums.
  2. Attention per (batch, q-chunk): scores^T [k,q] with K=64 matmuls
     packed two-per-array via row strips (head A partitions 0-63, head B
     64-127); exp on ACT (scale=1/8, no max subtraction -- scores are
     N(0,1)); PV+sums in one matmul stream via [ones|vw] lhsT; normalize
     with reciprocal_approx_fast. ACT is saturated here; projections for
     the other batch and the first batch's output projection fill the
     tensor-engine slack.
  3. Two 8-way AllToAlls (one per batch). Batch 0's fires mid-kernel and
     hides under batch-1 attention; only batch 1's is tail-exposed.
  4. Output projection per batch: bias first (start=True ones-matmul,
     PSUM-resident during the A2A wait), then 8 contraction steps over
     the gathered [1024ch, 256q] slice.
Host assembles the 8 disjoint [2, 256, 1024] slices.
"""
import contextlib
import ctypes
import os
import sys
import types

import ml_dtypes
import numpy as np

for _p in ("/opt/trn_rl_repo", os.path.expanduser("~/.axon_site/_ro/trn_rl_repo")):
    if os.path.isdir(_p) and _p not in sys.path:
        sys.path.insert(0, _p)
        break


def _install_ntff_hook():
    """run_bass_kernel_spmd(trace=True) under axon imports antenv.axon_hooks,
    which this image lacks; provide it so tracing degrades gracefully."""
    if "antenv.axon_hooks" in sys.modules:
        return
    mod = types.ModuleType("antenv.axon_hooks")
    state = {"hook": None}
    mod.set_axon_ntff_profile_hook = lambda h: state.__setitem__("hook", h)
    mod.get_axon_ntff_profile_hook = lambda: state["hook"]
    sys.modules["antenv.axon_hooks"] = mod
    try:
        import antenv

        antenv.axon_hooks = mod
    except ImportError:
        pass
    so_path = "/opt/axon/libaxon_pjrt.so"
    try:
        lib = ctypes.CDLL(so_path)
        if not hasattr(lib, "axon_start_nrt_profile"):
            return
        lib.axon_start_nrt_profile.argtypes = [
            ctypes.POINTER(ctypes.c_int64), ctypes.c_size_t]
        lib.axon_start_nrt_profile.restype = ctypes.c_int64
        lib.axon_stop_nrt_profile.argtypes = [ctypes.c_char_p]
        lib.axon_stop_nrt_profile.restype = ctypes.c_int64

        @contextlib.contextmanager
        def _ctx(output_dir, device_ids):
            import jax

            jax.devices()
            if device_ids:
                ids = (ctypes.c_int64 * len(device_ids))(*device_ids)
                rc = lib.axon_start_nrt_profile(ids, len(device_ids))
            else:
                rc = lib.axon_start_nrt_profile(None, 0)
            if rc != 0:
                raise RuntimeError(f"axon_start_nrt_profile rc={rc}")
            try:
                yield
            finally:
                n = lib.axon_stop_nrt_profile(str(output_dir).encode())
                print(f"profile: {n} ntff file(s) in {output_dir}",
                      file=sys.stderr)

        state["hook"] = _ctx
    except OSError:
        pass


_install_ntff_hook()

import concourse.bacc as bacc  # noqa: E402
import concourse.mybir as mybir  # noqa: E402
import concourse.tile as tile  # noqa: E402
from concourse.bass_utils import run_bass_kernel_spmd  # noqa: E402

F32 = mybir.dt.float32
BF16 = mybir.dt.bfloat16
AF = mybir.ActivationFunctionType
MUL = mybir.AluOpType.mult
ADD = mybir.AluOpType.add

N_CORES = 8
B, S, D, H, HD = 2, 2048, 1024, 16, 64
DPC = 2 * HD       # 128 projection columns per core (2 heads)
NCH = 4            # q chunks of 512 per batch
QW = S // NCH      # 512
QO = 256           # output queries per (core, batch)
KT = S // 128      # 16 k-position tiles per batch
DKT = D // 128     # 8 d_model contraction tiles

_CACHED_NC = None


def _build():
    nc = bacc.Bacc("TRN2", target_bir_lowering=False, debug=False,
                   num_devices=N_CORES)

    # x tensors hold BOTH batches: chunk index cidx = b*4 + ch, arranged as
    # [cidx, partition(d_in%128), k-tile(d_in//128), seq] so chunk DMAs read
    # 8KB-contiguous runs per partition
    qT = nc.dram_tensor("qT", [2 * NCH, 128, DKT, QW], BF16,
                        kind="ExternalInput").ap()
    kT = nc.dram_tensor("kT", [2 * NCH, 128, DKT, QW], BF16,
                        kind="ExternalInput").ap()
    vT = nc.dram_tensor("vT", [2 * NCH, 128, DKT, QW], BF16,
                        kind="ExternalInput").ap()
    wq = nc.dram_tensor("wq", [128, DKT, DPC], BF16,
                        kind="ExternalInput").ap()
    wk = nc.dram_tensor("wk", [128, DKT, DPC], BF16,
                        kind="ExternalInput").ap()
    wv = nc.dram_tensor("wv", [128, DKT, DPC], BF16,
                        kind="ExternalInput").ap()
    bq1 = nc.dram_tensor("bq1", [128, 1], F32, kind="ExternalInput").ap()
    bk1 = nc.dram_tensor("bk1", [128, 1], F32, kind="ExternalInput").ap()
    bv1 = nc.dram_tensor("bv1", [128, 1], F32, kind="ExternalInput").ap()
    # Wo rows permuted to the gathered-channel order: row (j*128 + dh*64 + d)
    # holds Wo[(2j+dh)*64 + d, :]
    wo1 = nc.dram_tensor("wo1", [128, DKT, D], BF16,
                         kind="ExternalInput").ap()
    bo1 = nc.dram_tensor("bo1", [1, D], BF16, kind="ExternalInput").ap()
    ident = nc.dram_tensor("ident", [128, 128], BF16,
                           kind="ExternalInput").ap()
    out = nc.dram_tensor("out", [2, QO, D], BF16, kind="ExternalOutput").ap()

    taps = {}
    if os.environ.get("DEBUG_TAPS"):
        taps["tqwT"] = nc.dram_tensor("tqwT", [128, 2, S], BF16,
                                      kind="ExternalOutput").ap()
        taps["tkwT"] = nc.dram_tensor("tkwT", [128, 2, S], BF16,
                                      kind="ExternalOutput").ap()
        taps["tvwx"] = nc.dram_tensor("tvwx", [128, 2, KT, 256], BF16,
                                      kind="ExternalOutput").ap()

    with tile.TileContext(nc) as tc:
        with tc.tile_pool(name="xw", bufs=1) as xw, \
             tc.tile_pool(name="dram", bufs=1, space="DRAM") as dram:
            # long-lived tiles
            qwT = xw.tile([128, 2, S], BF16, name="qwT")   # [dh*64+d, b, q]
            kwT = xw.tile([128, 2, S], BF16, name="kwT")
            # [kpos%128, b, kt, (ones64|vw64) x2 dh]
            vwx = xw.tile([128, 2, KT, 256], BF16, name="vwx")
            bq_sb = xw.tile([128, 1], F32, name="bq_sb")
            bk_sb = xw.tile([128, 1], F32, name="bk_sb")
            bv_sb = xw.tile([128, 1], F32, name="bv_sb")
            onesb = xw.tile([1, 128], BF16, name="onesb")
            bo_sb = xw.tile([1, D], BF16, name="bo_sb")
            id_sb = xw.tile([128, 128], BF16, name="id_sb")
            wo_sb = xw.tile([128, DKT, D], BF16, name="wo_sb")
            gth0 = xw.tile([128, DKT, QO], BF16, name="gth0")
            gth1 = xw.tile([128, DKT, QO], BF16, name="gth1")
            wq_sb = xw.tile([128, DKT, DPC], BF16, name="wq_sb")
            wk_sb = xw.tile([128, DKT, DPC], BF16, name="wk_sb")
            wv_sb = xw.tile([128, DKT, DPC], BF16, name="wv_sb")

            ones_f = xw.tile([1, 128], F32, name="ones_f")
            nc.gpsimd.memset(ones_f[:], 1.0)
            nc.vector.tensor_copy(onesb[:], ones_f[:])
            # static ones columns of vwx (softmax-denominator lhsT rows);
            # two 4D memsets (one per dh) keep the APs within dim limits
            nc.vector.memset(vwx[:, :, :, 0:64], 1.0)
            nc.vector.memset(vwx[:, :, :, 128:192], 1.0)

            # A2A staging: cin rows [(2ch+h)*128 + dh*64 + d] = ctx^T rows,
            # chunk j of 128 rows goes to core j (= q block [256j, 256j+256))
            cin0 = dram.tile([1024, QO], BF16, name="cin0")
            cout0 = dram.tile([1024, QO], BF16, name="cout0")
            cin1 = dram.tile([1024, QO], BF16, name="cin1")
            cout1 = dram.tile([1024, QO], BF16, name="cout1")
            cins, couts = (cin0, cin1), (cout0, cout1)
            # full-size scratch warmup exchange absorbs the NRT
            # first-collective setup cost off the critical path. The payload
            # MUST match the real A2As: a smaller warmup leaves the real
            # 512KB exchanges running 3-5x slower (NRT sizes channel state
            # from the first op), which also starves concurrent input DMA.
            ccw_in = dram.tile([1024, QO], BF16, name="ccw_in")
            ccw_out = dram.tile([1024, QO], BF16, name="ccw_out")

            # ---- startup DMAs, strict need-order -------------------------
            # first matmul needs kt chunk 0 + wk only: keep them unblocked.
            # Each hardware DMA queue sustains only ~115GB/s, so chunk halves
            # rotate across all three DMA-capable engines (sync/scalar/
            # gpsimd) to keep aggregate input bandwidth near the HBM limit.
            _xq = [nc.sync, nc.scalar, nc.gpsimd]
            _xqi = [0]

            def xchunk_dma(xtp, x_dram, cidx, name, qa=None, qb=None):
                t = xtp.tile([128, DKT, QW], BF16, name=name, tag="xt")
                h = DKT // 2
                if qa is None:
                    qa = _xq[_xqi[0] % 3]
                    qb = _xq[(_xqi[0] + 1) % 3]
                    _xqi[0] += 2
                qa.dma_start(out=t[:, 0:h, :], in_=x_dram[cidx][:, 0:h, :])
                qb.dma_start(out=t[:, h:DKT, :], in_=x_dram[cidx][:, h:DKT, :])
                return t

            with tc.tile_pool(name="xt", bufs=5) as xtp, \
                 tc.tile_pool(name="vts", bufs=2) as vts, \
                 tc.tile_pool(name="stg", bufs=8) as stp, \
                 tc.tile_pool(name="osb", bufs=2) as osb, \
                 tc.tile_pool(name="prp", bufs=36) as prp, \
                 tc.tile_pool(name="sps", bufs=2, space="PSUM") as sps, \
                 tc.tile_pool(name="vps", bufs=2, space="PSUM") as vps, \
                 tc.tile_pool(name="aps", bufs=2, space="PSUM") as aps:

                # x-chunk DMA emission must match consumption order exactly:
                # the 4-buffer ring makes chunk i's DMA wait on chunk i-4's
                # last reader, so out-of-order emission would deadlock the
                # in-order engines.
                def xchunk3(x_dram, cidx, name):
                    """Chunk split three ways across all DMA queues, in
                    strict consumption order -- every queue then delivers
                    chunks in the same order the PE consumes them."""
                    t = xtp.tile([128, DKT, QW], BF16, name=name, tag="xt")
                    nc.sync.dma_start(out=t[:, 0:3, :],
                                      in_=x_dram[cidx][:, 0:3, :])
                    nc.scalar.dma_start(out=t[:, 3:6, :],
                                        in_=x_dram[cidx][:, 3:6, :])
                    nc.gpsimd.dma_start(out=t[:, 6:8, :],
                                        in_=x_dram[cidx][:, 6:8, :])
                    return t

                kts, qts, vtss = {}, {}, {}
                nc.gpsimd.dma_start(out=wk_sb[:], in_=wk[:])
                kts[0] = xchunk3(kT, 0, "kt0")
                nc.gpsimd.dma_start(out=bk_sb[:], in_=bk1[:])
                nc.gpsimd.dma_start(out=bq_sb[:], in_=bq1[:])
                nc.gpsimd.dma_start(out=bv_sb[:], in_=bv1[:])
                nc.gpsimd.dma_start(out=bo_sb[:], in_=bo1[:])
                nc.gpsimd.dma_start(out=id_sb[:], in_=ident[:])
                kts[1] = xchunk3(kT, 1, "kt1")
                for c in range(2, 4):
                    kts[c] = xchunk_dma(xtp, kT, c, f"kt{c}")
                nc.gpsimd.dma_start(out=wq_sb[:], in_=wq[:])
                qts[0] = xchunk_dma(xtp, qT, 0, "qt0")
                # v chunks after q0: the first scores only need k+q data,
                # vw consumption starts a slot later
                nc.gpsimd.dma_start(out=wv_sb[:], in_=wv[:])
                for c in range(4):
                    vtss[c] = xchunk_dma(xtp, vT, c, f"vt{c}")
                # collective warmup fires now; CC setup cost retires long
                # before the first real A2A
                nc.gpsimd.collective_compute(
                    "AllToAll", mybir.AluOpType.bypass,
                    replica_groups=[list(range(N_CORES))],
                    ins=[ccw_in[:].opt()], outs=[ccw_out[:].opt()])
                qts[1] = xchunk_dma(xtp, qT, 1, "qt1")
                nc.gpsimd.dma_start(out=wo_sb[:], in_=wo1[:])
                # batch-1 chunk DMAs are emitted just-in-time inside the
                # attention loop: a dma_start blocks its issuing engine
                # until the tile pool slot frees, so emitting them all here
                # would stall the sync/gpsimd queues (delaying the stg
                # writes that feed the A2As)

                # in-loop chunk DMAs must avoid the scalar queue: a blocked
                # dma_start would stall the ACT engine mid-exp-stream
                _jq = [0]

                def dma(store, key, x_dram, name):
                    def fn():
                        qa, qb = ((nc.sync, nc.gpsimd) if _jq[0] % 2 == 0
                                  else (nc.gpsimd, nc.sync))
                        _jq[0] += 1
                        store[key] = xchunk_dma(xtp, x_dram, key, name,
                                                qa, qb)
                    return fn

                # ---- projection emitters --------------------------------
                def emit_qk(w_sb, b_sb, dstT, xt, b, ch):
                    """One chunk of qw^T/kw^T: [128 dout, 512 q] += bias."""
                    ps = aps.tile([128, QW], F32, name="ps", tag="ps")
                    for kk in range(DKT):
                        nc.tensor.matmul(ps[:], w_sb[:, kk, :], xt[:, kk, :],
                                         start=(kk == 0), stop=(kk == DKT - 1))
                    nc.vector.tensor_scalar_add(
                        dstT[:, b, ch * QW:(ch + 1) * QW], ps[:],
                        b_sb[:, 0:1])

                def emit_vw(xt, b, ch):
                    """One chunk of vw: project transposed (N=512), add bias
                    per-partition, then PE-transpose 128x128 blocks into the
                    natural [kpos, ch] slots of vwx."""
                    ps = aps.tile([128, QW], F32, name="ps", tag="ps")
                    for kk in range(DKT):
                        nc.tensor.matmul(ps[:], wv_sb[:, kk, :], xt[:, kk, :],
                                         start=(kk == 0), stop=(kk == DKT - 1))
                    vt_sb = vts.tile([128, QW], BF16, name="vt_sb", tag="vt")
                    nc.vector.tensor_scalar_add(vt_sb[:], ps[:], bv_sb[:, 0:1])
                    for s in range(4):
                        kt = ch * 4 + s
                        tp = vps.tile([128, QW], F32, name="pv", tag="pv")
                        tpb = tp[:].bitcast(BF16)[:, 0:128]
                        nc.tensor.transpose(
                            tpb, vt_sb[:, s * 128:(s + 1) * 128], id_sb[:])
                        dst = vwx[:, b, kt, :].rearrange(
                            "p (d c) -> p d c", d=2)
                        nc.vector.tensor_copy(
                            dst[:, :, 64:128],
                            tpb.rearrange("p (d c) -> p d c", d=2))

                # ---- attention emitters ---------------------------------
                def emit_scores(b, ch, kt_range):
                    prs = []
                    for kt in kt_range:
                        sq = sps.tile([128, 2, QW], F32, name="sq", tag="sq")
                        for dh in range(2):
                            nc.tensor.matmul(
                                sq[:, dh, :],
                                kwT[dh * 64:(dh + 1) * 64, b,
                                    kt * 128:(kt + 1) * 128],
                                qwT[dh * 64:(dh + 1) * 64, b,
                                    ch * QW:(ch + 1) * QW],
                                start=True, stop=True)
                        pr = prp.tile([128, 2, QW], BF16, name="pr", tag="pr")
                        nc.scalar.activation(pr[:], sq[:], AF.Exp, scale=0.125)
                        prs.append(pr)
                    return prs

                def emit_pvs(b, ch, prs, dhs=(0, 1)):
                    for dh in dhs:
                        pv = vps.tile([128, QW], F32, name="pv", tag="pv")
                        for kt in range(KT):
                            nc.tensor.matmul(
                                pv[:],
                                vwx[:, b, kt, dh * 128:(dh + 1) * 128],
                                prs[kt][:, dh, :],
                                start=(kt == 0), stop=(kt == KT - 1))
                        # sums land at PSUM rows 0:64 (ones first in lhsT)
                        rec = stp.tile([64, QW], F32, name="rec", tag="rec")
                        nc.vector.reciprocal_approx_fast(rec[:], pv[0:64, :])
                        stg = stp.tile([64, QW], BF16, name="stg", tag="stg")
                        nc.vector.tensor_tensor(stg[:], pv[64:128, :], rec[:],
                                                MUL)
                        for h, qeng in ((0, nc.sync), (1, nc.gpsimd)):
                            r0 = (2 * ch + h) * 128 + dh * 64
                            qeng.dma_start(
                                out=cins[b][r0:r0 + 64, :],
                                in_=stg[:, h * QO:(h + 1) * QO])

                def emit_pv_pair(b, ch, prs):
                    """Both heads' PV chains interleaved per k-tile: after
                    the slot's last exp, only ~2 matmuls remain instead of a
                    full 16-matmul chain. Used for the final slot so the
                    tail A2A triggers as early as possible."""
                    pvs = [vps.tile([128, QW], F32, name="pv", tag="pv")
                           for _ in range(2)]
                    for kt in range(KT):
                        for dh in range(2):
                            nc.tensor.matmul(
                                pvs[dh][:],
                                vwx[:, b, kt, dh * 128:(dh + 1) * 128],
                                prs[kt][:, dh, :],
                                start=(kt == 0), stop=(kt == KT - 1))
                    for dh in range(2):
                        pv = pvs[dh]
                        rec = stp.tile([64, QW], F32, name="rec", tag="rec")
                        nc.vector.reciprocal_approx_fast(rec[:], pv[0:64, :])
                        stg = stp.tile([64, QW], BF16, name="stg", tag="stg")
                        nc.vector.tensor_tensor(stg[:], pv[64:128, :], rec[:],
                                                MUL)
                        for h, qeng in ((0, nc.sync), (1, nc.gpsimd)):
                            r0 = (2 * ch + h) * 128 + dh * 64
                            qeng.dma_start(
                                out=cins[b][r0:r0 + 64, :],
                                in_=stg[:, h * QO:(h + 1) * QO])

                def emit_a2a(b):
                    nc.gpsimd.collective_compute(
                        "AllToAll", mybir.AluOpType.bypass,
                        replica_groups=[list(range(N_CORES))],
                        ins=[cins[b][:].opt()],
                        outs=[couts[b][:].opt()])

                def emit_gth(gth, cout, qa, qb):
                    src = cout.rearrange("(k p) n -> p k n", p=128)
                    qa.dma_start(out=gth[:, 0:DKT // 2, :],
                                 in_=src[:, 0:DKT // 2, :])
                    qb.dma_start(out=gth[:, DKT // 2:DKT, :],
                                 in_=src[:, DKT // 2:DKT, :])

                def emit_oproj(b, gth, mb, qs=None):
                    """Output projection for q-block mb of batch b; each
                    512-column half stores out as soon as its copy lands."""
                    if qs is None:
                        qs = (nc.sync, nc.sync)
                    for nch in range(2):
                        ps = aps.tile([128, QW], F32, name="ps", tag="ps")
                        nc.tensor.matmul(ps[:], onesb[:],
                                         bo_sb[:, nch * QW:(nch + 1) * QW],
                                         start=True, stop=False)
                        for kk in range(DKT):
                            nc.tensor.matmul(
                                ps[:], gth[:, kk, mb * 128:(mb + 1) * 128],
                                wo_sb[:, kk, nch * QW:(nch + 1) * QW],
                                start=False, stop=(kk == DKT - 1))
                        osb_t = osb.tile([128, QW], BF16, name="osb_t",
                                         tag="osb")
                        nc.vector.tensor_copy(osb_t[:], ps[:])
                        qs[nch].dma_start(
                            out=out[b, mb * 128:(mb + 1) * 128,
                                    nch * QW:(nch + 1) * QW],
                            in_=osb_t[:])

                # ---- phase 1a: batch-0 k/q projections ------------------
                # only what the first scores read; vw-b0 slides into slot
                # (0, 0) between the score groups (before that slot's PV,
                # which reads all of vwx batch 0 -- reads must follow
                # writes in trace order or the in-order engines deadlock).
                for ch in range(4):
                    emit_qk(wk_sb, bk_sb, kwT, kts[ch], 0, ch)
                emit_qk(wq_sb, bq_sb, qwT, qts[0], 0, 0)

                # ---- attention with aux work threaded through -----------
                # aux_mid[(b, ch)] is emitted between the slot's two score
                # groups, aux[(b, ch)] after its dh0 PV; everything a later
                # slot's scores/PVs read is emitted ahead of its first use.
                aux_mid = {
                    (0, 0): [lambda: emit_vw(vtss[0], 0, 0),
                             lambda: emit_vw(vtss[1], 0, 1)],
                }
                aux_pre = {
                    (0, 0): [lambda: emit_vw(vtss[2], 0, 2),
                             lambda: emit_vw(vtss[3], 0, 3)],
                }
                aux = {
                    (0, 0): [lambda: emit_qk(wq_sb, bq_sb, qwT, qts[1], 0, 1),
                             dma(qts, 2, qT, "qt2"),
                             dma(vtss, 4, vT, "vt4")],
                    (0, 1): [lambda: emit_qk(wq_sb, bq_sb, qwT, qts[2], 0, 2),
                             lambda: emit_vw(vtss[4], 1, 0),
                             dma(qts, 3, qT, "qt3"),
                             dma(vtss, 5, vT, "vt5"),
                             dma(vtss, 6, vT, "vt6")],
                    (0, 2): [lambda: emit_qk(wq_sb, bq_sb, qwT, qts[3], 0, 3),
                             lambda: emit_vw(vtss[5], 1, 1),
                             lambda: emit_vw(vtss[6], 1, 2),
                             dma(vtss, 7, vT, "vt7"),
                             dma(kts, 4, kT, "kt4"),
                             dma(kts, 5, kT, "kt5"),
                             dma(kts, 6, kT, "kt6"),
                             dma(kts, 7, kT, "kt7")],
                    (0, 3): [lambda: emit_vw(vtss[7], 1, 3),
                             dma(qts, 4, qT, "qt4"),
                             lambda: emit_qk(wk_sb, bk_sb, kwT, kts[4], 1, 0),
                             lambda: emit_qk(wk_sb, bk_sb, kwT, kts[5], 1, 1),
                             lambda: emit_qk(wk_sb, bk_sb, kwT, kts[6], 1, 2),
                             lambda: emit_qk(wk_sb, bk_sb, kwT, kts[7], 1, 3),
                             lambda: emit_qk(wq_sb, bq_sb, qwT, qts[4], 1, 0),
                             dma(qts, 5, qT, "qt5"),
                             dma(qts, 6, qT, "qt6")],
                    (1, 0): [lambda: emit_qk(wq_sb, bq_sb, qwT, qts[5], 1, 1),
                             dma(qts, 7, qT, "qt7")],
                    # gth0 loads at (1,1), both halves on gpsimd: it must be
                    # resident BEFORE A2A-1 starts (regular DMA queues
                    # starve to ~11GB/s while a collective is in flight, and
                    # a read of a collective output scheduled after A2A-1's
                    # trigger gets a wait on A2A-1 itself). gpsimd may block
                    # until A2A-0 completes; only stg h1 halves queue behind
                    # it, and those aren't needed until the A2A-1 trigger.
                    (1, 1): [lambda: emit_qk(wq_sb, bq_sb, qwT, qts[6], 1, 2),
                             lambda: emit_gth(gth0, cout0, nc.gpsimd,
                                              nc.gpsimd)],
                    (1, 2): [lambda: emit_qk(wq_sb, bq_sb, qwT, qts[7], 1, 3)],
                    (1, 3): [],
                }
                # the last slot of each batch interleaves both PV chains and
                # fires that batch's A2A immediately: batch 0's exchange
                # then hides under batch-1 attention, batch 1's tail
                # exposure starts as early as possible
                pair_slots = {(0, NCH - 1), (1, NCH - 1)}
                pend = None
                for b in range(2):
                    for ch in range(NCH):
                        prs = emit_scores(b, ch, range(KT // 2))
                        if pend is not None:
                            emit_pvs(*pend, dhs=(1,))
                            pend = None
                        for fn in aux_mid.get((b, ch), ()):
                            fn()
                        prs += emit_scores(b, ch, range(KT // 2, KT))
                        for fn in aux_pre.get((b, ch), ()):
                            fn()
                        if (b, ch) in pair_slots:
                            emit_pv_pair(b, ch, prs)
                            emit_a2a(b)
                        else:
                            emit_pvs(b, ch, prs, dhs=(0,))
                            pend = (b, ch, prs)
                        for fn in aux[(b, ch)]:
                            fn()

                if taps:
                    nc.sync.dma_start(out=taps["tqwT"][:], in_=qwT[:])
                    nc.sync.dma_start(out=taps["tkwT"][:], in_=kwT[:])
                    nc.sync.dma_start(out=taps["tvwx"][:], in_=vwx[:])

                # ---- tail ----------------------------------------------
                # batch-0 out-projection runs here, AFTER the A2A-1 trigger:
                # it covers the collective's latency with real work (the
                # attention slots it vacated were tensor-bound, so the
                # trigger also fires earlier); warm matmuls bridge any
                # remaining wait so batch-1's projection starts at full
                # clock.
                emit_oproj(0, gth0, 0)
                emit_oproj(0, gth0, 1)
                warm = aps.tile([128, QW], F32, name="ps", tag="ps")
                for i in range(26):
                    nc.tensor.matmul(warm[:], onesb[:], bo_sb[:, 0:QW],
                                     start=(i == 0), stop=(i == 25))
                # gth1 loads post-collective at full rate; the scalar queue
                # is idle once the exp stream ends, so use all three queues
                src1 = cout1.rearrange("(k p) n -> p k n", p=128)
                nc.sync.dma_start(out=gth1[:, 0:3, :], in_=src1[:, 0:3, :])
                nc.scalar.dma_start(out=gth1[:, 3:6, :], in_=src1[:, 3:6, :])
                nc.gpsimd.dma_start(out=gth1[:, 6:8, :], in_=src1[:, 6:8, :])
                emit_oproj(1, gth1, 0)
                emit_oproj(1, gth1, 1)

    nc.compile()
    return nc


def _get_nc():
    global _CACHED_NC
    if _CACHED_NC is None:
        _CACHED_NC = _build()
    return _CACHED_NC


def kernel(q, k, v, Wq, bq, Wk, bk, Wv, bv, Wo, bo, _return_results=False):
    q, k, v = (np.asarray(x, np.float32) for x in (q, k, v))
    Wq, bq, Wk, bk, Wv, bv, Wo, bo = (
        np.asarray(x, np.float32) for x in (Wq, bq, Wk, bk, Wv, bv, Wo, bo))

    nc = _get_nc()

    def arrange(x):  # [B, S, D] -> [2*NCH, 128, DKT, QW], same for all cores
        per_b = [np.ascontiguousarray(
            x[b].T.reshape(DKT, 128, NCH, QW).transpose(2, 1, 0, 3))
            for b in range(B)]
        return np.concatenate(per_b, axis=0).astype(ml_dtypes.bfloat16)

    qA, kA, vA = arrange(q), arrange(k), arrange(v)

    # Wo rows permuted to gathered-channel order (same for all cores)
    perm = np.empty(D, np.int64)
    for j in range(8):
        for dh in range(2):
            for d0 in range(64):
                perm[j * 128 + dh * 64 + d0] = (2 * j + dh) * 64 + d0
    woA = np.ascontiguousarray(
        Wo[perm].reshape(DKT, 128, D).transpose(1, 0, 2)).astype(
        ml_dtypes.bfloat16)
    identA = np.eye(128, dtype=np.float32).astype(ml_dtypes.bfloat16)
    boA = bo.reshape(1, D).astype(ml_dtypes.bfloat16)

    def warrange(w):  # [D, n] -> [128, DKT, n]
        n = w.shape[1]
        return np.ascontiguousarray(
            w.reshape(DKT, 128, n).transpose(1, 0, 2)).astype(
            ml_dtypes.bfloat16)

    in_maps = []
    for c in range(N_CORES):
        cols = slice(c * DPC, (c + 1) * DPC)
        in_maps.append({
            "qT": qA, "kT": kA, "vT": vA,
            "wq": warrange(Wq[:, cols]),
            "wk": warrange(Wk[:, cols]),
            "wv": warrange(Wv[:, cols]),
            "bq1": np.ascontiguousarray(bq[cols].reshape(128, 1)),
            "bk1": np.ascontiguousarray(bk[cols].reshape(128, 1)),
            "bv1": np.ascontiguousarray(bv[cols].reshape(128, 1)),
            "wo1": woA, "bo1": boA, "ident": identA,
        })

    res = run_bass_kernel_spmd(nc, in_maps, core_ids=list(range(N_CORES)))

    full = np.empty((B, S, D), np.float32)
    for c in range(N_CORES):
        o = np.asarray(res.results[c]["out"], ml_dtypes.bfloat16)
        for b in range(B):
            full[b, c * QO:(c + 1) * QO] = o[b].astype(np.float32)
    if _return_results:
        return full, res
    return full


# revision 58
# speedup vs baseline: 1.0253x; 1.0253x over previous
"""Multi-head attention (B=2, S=2048, D=1024, H=16) on 8 Trainium2 cores.

Sharding: head x batch. Core c handles heads {2c, 2c+1} for BOTH batches
(instead of 4 heads x 1 batch). This makes the post-attention exchange a
clean 8-way AllToAll per batch with zero redundancy: core c sends its
[128ch, 512q] blocks and receives exactly its [1024ch, 256q] output slice
(core c owns queries [256c, 256c+256) of each batch). Compared to the
batch-split sharding this halves the A2A payload (2 x 512KB, batch-0's
exchange hidden under batch-1 attention) and halves the output projection
(contract 1024 real channels, no zero-padded half).

Per core:
  1. Projections, interleaved with attention so the PE never idles:
     qw^T/kw^T in transposed layout (bf16 -- full-rate scores matmuls and
     half-size LDWEIGHTS vs fp32r), bias fused via DVE per-partition add;
     vw first as vw^T (N=512 matmuls, per-partition DVE bias), then PE
     transposes [128,128] blocks into the natural [kpos, ch] layout the
     PV matmul needs, with static memset ones columns for the fused
     softmax-denominator sums.
  2. Attention per (batch, q-chunk): scores^T [k,q] with K=64 matmuls
     packed two-per-array via row strips (head A partitions 0-63, head B
     64-127); exp on ACT (scale=1/8, no max subtraction -- scores are
     N(0,1)); PV+sums in one matmul stream via [ones|vw] lhsT; normalize
     with reciprocal_approx_fast. ACT is saturated here; projections for
     the other batch and the first batch's output projection fill the
     tensor-engine slack.
  3. Two 8-way AllToAlls (one per batch). Batch 0's fires mid-kernel and
     hides under batch-1 attention; only batch 1's is tail-exposed.
  4. Output projection per batch: bias first (start=True ones-matmul,
     PSUM-resident during the A2A wait), then 8 contraction steps over
     the gathered [1024ch, 256q] slice.
Host assembles the 8 disjoint [2, 256, 1024] slices.
"""
import contextlib
import ctypes
import os
import sys
import types

import ml_dtypes
import numpy as np

for _p in ("/opt/trn_rl_repo", os.path.expanduser("~/.axon_site/_ro/trn_rl_repo")):
    if os.path.isdir(_p) and _p not in sys.path:
        sys.path.insert(0, _p)
        break


def _install_ntff_hook():
    """run_bass_kernel_spmd(trace=True) under axon imports antenv.axon_hooks,
    which this image lacks; provide it so tracing degrades gracefully."""
    if "antenv.axon_hooks" in sys.modules:
        return
    mod = types.ModuleType("antenv.axon_hooks")
    state = {"hook": None}
    mod.set_axon_ntff_profile_hook = lambda h: state.__setitem__("hook", h)
    mod.get_axon_ntff_profile_hook = lambda: state["hook"]
    sys.modules["antenv.axon_hooks"] = mod
    try:
        import antenv

        antenv.axon_hooks = mod
    except ImportError:
        pass
    so_path = "/opt/axon/libaxon_pjrt.so"
    try:
        lib = ctypes.CDLL(so_path)
        if not hasattr(lib, "axon_start_nrt_profile"):
            return
        lib.axon_start_nrt_profile.argtypes = [
            ctypes.POINTER(ctypes.c_int64), ctypes.c_size_t]
        lib.axon_start_nrt_profile.restype = ctypes.c_int64
        lib.axon_stop_nrt_profile.argtypes = [ctypes.c_char_p]
        lib.axon_stop_nrt_profile.restype = ctypes.c_int64

        @contextlib.contextmanager
        def _ctx(output_dir, device_ids):
            import jax

            jax.devices()
            if device_ids:
                ids = (ctypes.c_int64 * len(device_ids))(*device_ids)
                rc = lib.axon_start_nrt_profile(ids, len(device_ids))
            else:
                rc = lib.axon_start_nrt_profile(None, 0)
            if rc != 0:
                raise RuntimeError(f"axon_start_nrt_profile rc={rc}")
            try:
                yield
            finally:
                n = lib.axon_stop_nrt_profile(str(output_dir).encode())
                print(f"profile: {n} ntff file(s) in {output_dir}",
                      file=sys.stderr)

        state["hook"] = _ctx
    except OSError:
        pass


_install_ntff_hook()

import concourse.bacc as bacc  # noqa: E402
import concourse.mybir as mybir  # noqa: E402
import concourse.tile as tile  # noqa: E402
from concourse.bass_utils import run_bass_kernel_spmd  # noqa: E402

F32 = mybir.dt.float32
BF16 = mybir.dt.bfloat16
AF = mybir.ActivationFunctionType
MUL = mybir.AluOpType.mult
ADD = mybir.AluOpType.add

N_CORES = 8
B, S, D, H, HD = 2, 2048, 1024, 16, 64
DPC = 2 * HD       # 128 projection columns per core (2 heads)
NCH = 4            # q chunks of 512 per batch
QW = S // NCH      # 512
QO = 256           # output queries per (core, batch)
KT = S // 128      # 16 k-position tiles per batch
DKT = D // 128     # 8 d_model contraction tiles

_CACHED_NC = None


def _build():
    nc = bacc.Bacc("TRN2", target_bir_lowering=False, debug=False,
                   num_devices=N_CORES)

    # x tensors hold BOTH batches: chunk index cidx = b*4 + ch, arranged as
    # [cidx, partition(d_in%128), k-tile(d_in//128), seq] so chunk DMAs read
    # 8KB-contiguous runs per partition
    qT = nc.dram_tensor("qT", [2 * NCH, 128, DKT, QW], BF16,
                        kind="ExternalInput").ap()
    kT = nc.dram_tensor("kT", [2 * NCH, 128, DKT, QW], BF16,
                        kind="ExternalInput").ap()
    vT = nc.dram_tensor("vT", [2 * NCH, 128, DKT, QW], BF16,
                        kind="ExternalInput").ap()
    wq = nc.dram_tensor("wq", [128, DKT, DPC], BF16,
                        kind="ExternalInput").ap()
    wk = nc.dram_tensor("wk", [128, DKT, DPC], BF16,
                        kind="ExternalInput").ap()
    wv = nc.dram_tensor("wv", [128, DKT, DPC], BF16,
                        kind="ExternalInput").ap()
    bq1 = nc.dram_tensor("bq1", [128, 1], F32, kind="ExternalInput").ap()
    bk1 = nc.dram_tensor("bk1", [128, 1], F32, kind="ExternalInput").ap()
    bv1 = nc.dram_tensor("bv1", [128, 1], F32, kind="ExternalInput").ap()
    # Wo rows permuted to the gathered-channel order: row (j*128 + dh*64 + d)
    # holds Wo[(2j+dh)*64 + d, :]
    wo1 = nc.dram_tensor("wo1", [128, DKT, D], BF16,
                         kind="ExternalInput").ap()
    bo1 = nc.dram_tensor("bo1", [1, D], BF16, kind="ExternalInput").ap()
    ident = nc.dram_tensor("ident", [128, 128], BF16,
                           kind="ExternalInput").ap()
    out = nc.dram_tensor("out", [2, QO, D], BF16, kind="ExternalOutput").ap()

    taps = {}
    if os.environ.get("DEBUG_TAPS"):
        taps["tqwT"] = nc.dram_tensor("tqwT", [128, 2, S], BF16,
                                      kind="ExternalOutput").ap()
        taps["tkwT"] = nc.dram_tensor("tkwT", [128, 2, S], BF16,
                                      kind="ExternalOutput").ap()
        taps["tvwx"] = nc.dram_tensor("tvwx", [128, 2, KT, 256], BF16,
                                      kind="ExternalOutput").ap()

    with tile.TileContext(nc) as tc:
        with tc.tile_pool(name="xw", bufs=1) as xw, \
             tc.tile_pool(name="dram", bufs=1, space="DRAM") as dram:
            # long-lived tiles
            qwT = xw.tile([128, 2, S], BF16, name="qwT")   # [dh*64+d, b, q]
            kwT = xw.tile([128, 2, S], BF16, name="kwT")
            # [kpos%128, b, kt, (ones64|vw64) x2 dh]
            vwx = xw.tile([128, 2, KT, 256], BF16, name="vwx")
            bq_sb = xw.tile([128, 1], F32, name="bq_sb")
            bk_sb = xw.tile([128, 1], F32, name="bk_sb")
            bv_sb = xw.tile([128, 1], F32, name="bv_sb")
            onesb = xw.tile([1, 128], BF16, name="onesb")
            bo_sb = xw.tile([1, D], BF16, name="bo_sb")
            id_sb = xw.tile([128, 128], BF16, name="id_sb")
            wo_sb = xw.tile([128, DKT, D], BF16, name="wo_sb")
            gth0 = xw.tile([128, DKT, QO], BF16, name="gth0")
            gth1 = xw.tile([128, DKT, QO], BF16, name="gth1")
            wq_sb = xw.tile([128, DKT, DPC], BF16, name="wq_sb")
            wk_sb = xw.tile([128, DKT, DPC], BF16, name="wk_sb")
            wv_sb = xw.tile([128, DKT, DPC], BF16, name="wv_sb")

            ones_f = xw.tile([1, 128], F32, name="ones_f")
            nc.gpsimd.memset(ones_f[:], 1.0)
            nc.vector.tensor_copy(onesb[:], ones_f[:])
            # static ones columns of vwx (softmax-denominator lhsT rows);
            # two 4D memsets (one per dh) keep the APs within dim limits
            nc.vector.memset(vwx[:, :, :, 0:64], 1.0)
            nc.vector.memset(vwx[:, :, :, 128:192], 1.0)

            # A2A staging: cin rows [(2ch+h)*128 + dh*64 + d] = ctx^T rows,
            # chunk j of 128 rows goes to core j (= q block [256j, 256j+256))
            cin0 = dram.tile([1024, QO], BF16, name="cin0")
            cout0 = dram.tile([1024, QO], BF16, name="cout0")
            cin1 = dram.tile([1024, QO], BF16, name="cin1")
            cout1 = dram.tile([1024, QO], BF16, name="cout1")
            cins, couts = (cin0, cin1), (cout0, cout1)
            # full-size scratch warmup exchange absorbs the NRT
            # first-collective setup cost off the critical path. The payload
            # MUST match the real A2As: a smaller warmup leaves the real
            # 512KB exchanges running 3-5x slower (NRT sizes channel state
            # from the first op), which also starves concurrent input DMA.
            ccw_in = dram.tile([1024, QO], BF16, name="ccw_in")
            ccw_out = dram.tile([1024, QO], BF16, name="ccw_out")

            # ---- startup DMAs, strict need-order -------------------------
            # first matmul needs kt chunk 0 + wk only: keep them unblocked.
            # Each hardware DMA queue sustains only ~115GB/s, so chunk halves
            # rotate across all three DMA-capable engines (sync/scalar/
            # gpsimd) to keep aggregate input bandwidth near the HBM limit.
            _xq = [nc.sync, nc.scalar, nc.gpsimd]
            _xqi = [0]

            def xchunk_dma(xtp, x_dram, cidx, name, qa=None, qb=None):
                t = xtp.tile([128, DKT, QW], BF16, name=name, tag="xt")
                h = DKT // 2
                if qa is None:
                    qa = _xq[_xqi[0] % 3]
                    qb = _xq[(_xqi[0] + 1) % 3]
                    _xqi[0] += 2
                qa.dma_start(out=t[:, 0:h, :], in_=x_dram[cidx][:, 0:h, :])
                qb.dma_start(out=t[:, h:DKT, :], in_=x_dram[cidx][:, h:DKT, :])
                return t

            with tc.tile_pool(name="xt", bufs=5) as xtp, \
                 tc.tile_pool(name="vts", bufs=2) as vts, \
                 tc.tile_pool(name="stg", bufs=8) as stp, \
                 tc.tile_pool(name="osb", bufs=2) as osb, \
                 tc.tile_pool(name="prp", bufs=36) as prp, \
                 tc.tile_pool(name="sps", bufs=2, space="PSUM") as sps, \
                 tc.tile_pool(name="vps", bufs=2, space="PSUM") as vps, \
                 tc.tile_pool(name="aps", bufs=2, space="PSUM") as aps:

                # x-chunk DMA emission must match consumption order exactly:
                # the 4-buffer ring makes chunk i's DMA wait on chunk i-4's
                # last reader, so out-of-order emission would deadlock the
                # in-order engines.
                def xchunk3(x_dram, cidx, name):
                    """Chunk split three ways across all DMA queues, in
                    strict consumption order -- every queue then delivers
                    chunks in the same order the PE consumes them."""
                    t = xtp.tile([128, DKT, QW], BF16, name=name, tag="xt")
                    nc.sync.dma_start(out=t[:, 0:3, :],
                                      in_=x_dram[cidx][:, 0:3, :])
                    nc.scalar.dma_start(out=t[:, 3:6, :],
                                        in_=x_dram[cidx][:, 3:6, :])
                    nc.gpsimd.dma_start(out=t[:, 6:8, :],
                                        in_=x_dram[cidx][:, 6:8, :])
                    return t

                kts, qts, vtss = {}, {}, {}
                nc.gpsimd.dma_start(out=wk_sb[:], in_=wk[:])
                kts[0] = xchunk3(kT, 0, "kt0")
                nc.gpsimd.dma_start(out=bk_sb[:], in_=bk1[:])
                nc.gpsimd.dma_start(out=bq_sb[:], in_=bq1[:])
                nc.gpsimd.dma_start(out=bv_sb[:], in_=bv1[:])
                nc.gpsimd.dma_start(out=bo_sb[:], in_=bo1[:])
                nc.gpsimd.dma_start(out=id_sb[:], in_=ident[:])
                kts[1] = xchunk3(kT, 1, "kt1")
                for c in range(2, 4):
                    kts[c] = xchunk_dma(xtp, kT, c, f"kt{c}")
                nc.gpsimd.dma_start(out=wq_sb[:], in_=wq[:])
                qts[0] = xchunk_dma(xtp, qT, 0, "qt0")
                # v chunks after q0: the first scores only need k+q data,
                # vw consumption starts a slot later
                nc.gpsimd.dma_start(out=wv_sb[:], in_=wv[:])
                for c in range(4):
                    vtss[c] = xchunk_dma(xtp, vT, c, f"vt{c}")
                # collective warmup fires now; CC setup cost retires long
                # before the first real A2A
                nc.gpsimd.collective_compute(
                    "AllToAll", mybir.AluOpType.bypass,
                    replica_groups=[list(range(N_CORES))],
                    ins=[ccw_in[:].opt()], outs=[ccw_out[:].opt()])
                qts[1] = xchunk_dma(xtp, qT, 1, "qt1")
                nc.gpsimd.dma_start(out=wo_sb[:], in_=wo1[:])
                # batch-1 chunk DMAs are emitted just-in-time inside the
                # attention loop: a dma_start blocks its issuing engine
                # until the tile pool slot frees, so emitting them all here
                # would stall the sync/gpsimd queues (delaying the stg
                # writes that feed the A2As)

                # in-loop chunk DMAs must avoid the scalar queue: a blocked
                # dma_start would stall the ACT engine mid-exp-stream
                _jq = [0]

                def dma(store, key, x_dram, name):
                    def fn():
                        qa, qb = ((nc.sync, nc.gpsimd) if _jq[0] % 2 == 0
                                  else (nc.gpsimd, nc.sync))
                        _jq[0] += 1
                        store[key] = xchunk_dma(xtp, x_dram, key, name,
                                                qa, qb)
                    return fn

                # ---- projection emitters --------------------------------
                def emit_qk(w_sb, b_sb, dstT, xt, b, ch):
                    """One chunk of qw^T/kw^T: [128 dout, 512 q] += bias."""
                    ps = aps.tile([128, QW], F32, name="ps", tag="ps")
                    for kk in range(DKT):
                        nc.tensor.matmul(ps[:], w_sb[:, kk, :], xt[:, kk, :],
                                         start=(kk == 0), stop=(kk == DKT - 1))
                    nc.vector.tensor_scalar_add(
                        dstT[:, b, ch * QW:(ch + 1) * QW], ps[:],
                        b_sb[:, 0:1])

                def emit_vw(xt, b, ch):
                    """One chunk of vw: project transposed (N=512), add bias
                    per-partition, then PE-transpose 128x128 blocks into the
                    natural [kpos, ch] slots of vwx."""
                    ps = aps.tile([128, QW], F32, name="ps", tag="ps")
                    for kk in range(DKT):
                        nc.tensor.matmul(ps[:], wv_sb[:, kk, :], xt[:, kk, :],
                                         start=(kk == 0), stop=(kk == DKT - 1))
                    vt_sb = vts.tile([128, QW], BF16, name="vt_sb", tag="vt")
                    nc.vector.tensor_scalar_add(vt_sb[:], ps[:], bv_sb[:, 0:1])
                    for s in range(4):
                        kt = ch * 4 + s
                        tp = vps.tile([128, QW], F32, name="pv", tag="pv")
                        tpb = tp[:].bitcast(BF16)[:, 0:128]
                        nc.tensor.transpose(
                            tpb, vt_sb[:, s * 128:(s + 1) * 128], id_sb[:])
                        dst = vwx[:, b, kt, :].rearrange(
                            "p (d c) -> p d c", d=2)
                        nc.vector.tensor_copy(
                            dst[:, :, 64:128],
                            tpb.rearrange("p (d c) -> p d c", d=2))

                # ---- attention emitters ---------------------------------
                def emit_scores(b, ch, kt_range):
                    prs = []
                    for kt in kt_range:
                        sq = sps.tile([128, 2, QW], F32, name="sq", tag="sq")
                        for dh in range(2):
                            nc.tensor.matmul(
                                sq[:, dh, :],
                                kwT[dh * 64:(dh + 1) * 64, b,
                                    kt * 128:(kt + 1) * 128],
                                qwT[dh * 64:(dh + 1) * 64, b,
                                    ch * QW:(ch + 1) * QW],
                                start=True, stop=True)
                        pr = prp.tile([128, 2, QW], BF16, name="pr", tag="pr")
                        nc.scalar.activation(pr[:], sq[:], AF.Exp, scale=0.125)
                        prs.append(pr)
                    return prs

                def emit_pvs(b, ch, prs, dhs=(0, 1)):
                    for dh in dhs:
                        pv = vps.tile([128, QW], F32, name="pv", tag="pv")
                        for kt in range(KT):
                            nc.tensor.matmul(
                                pv[:],
                                vwx[:, b, kt, dh * 128:(dh + 1) * 128],
                                prs[kt][:, dh, :],
                                start=(kt == 0), stop=(kt == KT - 1))
                        # sums land at PSUM rows 0:64 (ones first in lhsT)
                        rec = stp.tile([64, QW], F32, name="rec", tag="rec")
                        nc.vector.reciprocal_approx_fast(rec[:], pv[0:64, :])
                        stg = stp.tile([64, QW], BF16, name="stg", tag="stg")
                        nc.vector.tensor_tensor(stg[:], pv[64:128, :], rec[:],
                                                MUL)
                        # batch-1 slots keep these off gpsimd: gth0's
                        # dma_start blocks that queue until A2A-0 completes,
                        # and cin writes stuck behind it stall the stg ring
                        # (and with it DVE -> PSUM -> PE) when the
                        # collective runs long
                        qpair = (((0, nc.sync), (1, nc.gpsimd)) if b == 0
                                 else ((0, nc.sync), (1, nc.sync)))
                        for h, qeng in qpair:
                            r0 = (2 * ch + h) * 128 + dh * 64
                            qeng.dma_start(
                                out=cins[b][r0:r0 + 64, :],
                                in_=stg[:, h * QO:(h + 1) * QO])

                def emit_pv_pair(b, ch, prs):
                    """Both heads' PV chains interleaved per k-tile: after
                    the slot's last exp, only ~2 matmuls remain instead of a
                    full 16-matmul chain. Used for the final slot so the
                    tail A2A triggers as early as possible."""
                    pvs = [vps.tile([128, QW], F32, name="pv", tag="pv")
                           for _ in range(2)]
                    for kt in range(KT):
                        for dh in range(2):
                            nc.tensor.matmul(
                                pvs[dh][:],
                                vwx[:, b, kt, dh * 128:(dh + 1) * 128],
                                prs[kt][:, dh, :],
                                start=(kt == 0), stop=(kt == KT - 1))
                    for dh in range(2):
                        pv = pvs[dh]
                        rec = stp.tile([64, QW], F32, name="rec", tag="rec")
                        nc.vector.reciprocal_approx_fast(rec[:], pv[0:64, :])
                        stg = stp.tile([64, QW], BF16, name="stg", tag="stg")
                        nc.vector.tensor_tensor(stg[:], pv[64:128, :], rec[:],
                                                MUL)
                        for h, qeng in ((0, nc.sync), (1, nc.gpsimd)):
                            r0 = (2 * ch + h) * 128 + dh * 64
                            qeng.dma_start(
                                out=cins[b][r0:r0 + 64, :],
                                in_=stg[:, h * QO:(h + 1) * QO])

                def emit_a2a(b):
                    nc.gpsimd.collective_compute(
                        "AllToAll", mybir.AluOpType.bypass,
                        replica_groups=[list(range(N_CORES))],
                        ins=[cins[b][:].opt()],
                        outs=[couts[b][:].opt()])

                def emit_gth(gth, cout, qa, qb):
                    src = cout.rearrange("(k p) n -> p k n", p=128)
                    qa.dma_start(out=gth[:, 0:DKT // 2, :],
                                 in_=src[:, 0:DKT // 2, :])
                    qb.dma_start(out=gth[:, DKT // 2:DKT, :],
                                 in_=src[:, DKT // 2:DKT, :])

                def emit_oproj(b, gth, mb, qs=None):
                    """Output projection for q-block mb of batch b; each
                    512-column half stores out as soon as its copy lands."""
                    if qs is None:
                        qs = (nc.sync, nc.sync)
                    for nch in range(2):
                        ps = aps.tile([128, QW], F32, name="ps", tag="ps")
                        nc.tensor.matmul(ps[:], onesb[:],
                                         bo_sb[:, nch * QW:(nch + 1) * QW],
                                         start=True, stop=False)
                        for kk in range(DKT):
                            nc.tensor.matmul(
                                ps[:], gth[:, kk, mb * 128:(mb + 1) * 128],
                                wo_sb[:, kk, nch * QW:(nch + 1) * QW],
                                start=False, stop=(kk == DKT - 1))
                        osb_t = osb.tile([128, QW], BF16, name="osb_t",
                                         tag="osb")
                        nc.vector.tensor_copy(osb_t[:], ps[:])
                        qs[nch].dma_start(
                            out=out[b, mb * 128:(mb + 1) * 128,
                                    nch * QW:(nch + 1) * QW],
                            in_=osb_t[:])

                # ---- phase 1a: batch-0 k/q projections ------------------
                # only what the first scores read; vw-b0 slides into slot
                # (0, 0) between the score groups (before that slot's PV,
                # which reads all of vwx batch 0 -- reads must follow
                # writes in trace order or the in-order engines deadlock).
                for ch in range(4):
                    emit_qk(wk_sb, bk_sb, kwT, kts[ch], 0, ch)
                emit_qk(wq_sb, bq_sb, qwT, qts[0], 0, 0)

                # ---- attention with aux work threaded through -----------
                # aux_mid[(b, ch)] is emitted between the slot's two score
                # groups, aux[(b, ch)] after its dh0 PV; everything a later
                # slot's scores/PVs read is emitted ahead of its first use.
                aux_mid = {
                    (0, 0): [lambda: emit_vw(vtss[0], 0, 0),
                             lambda: emit_vw(vtss[1], 0, 1)],
                }
                aux_pre = {
                    (0, 0): [lambda: emit_vw(vtss[2], 0, 2),
                             lambda: emit_vw(vtss[3], 0, 3)],
                }
                aux = {
                    (0, 0): [lambda: emit_qk(wq_sb, bq_sb, qwT, qts[1], 0, 1),
                             dma(qts, 2, qT, "qt2"),
                             dma(vtss, 4, vT, "vt4")],
                    (0, 1): [lambda: emit_qk(wq_sb, bq_sb, qwT, qts[2], 0, 2),
                             lambda: emit_vw(vtss[4], 1, 0),
                             dma(qts, 3, qT, "qt3"),
                             dma(vtss, 5, vT, "vt5"),
                             dma(vtss, 6, vT, "vt6")],
                    (0, 2): [lambda: emit_qk(wq_sb, bq_sb, qwT, qts[3], 0, 3),
                             lambda: emit_vw(vtss[5], 1, 1),
                             lambda: emit_vw(vtss[6], 1, 2),
                             dma(vtss, 7, vT, "vt7"),
                             dma(kts, 4, kT, "kt4"),
                             dma(kts, 5, kT, "kt5"),
                             dma(kts, 6, kT, "kt6"),
                             dma(kts, 7, kT, "kt7")],
                    (0, 3): [lambda: emit_vw(vtss[7], 1, 3),
                             dma(qts, 4, qT, "qt4"),
                             lambda: emit_qk(wk_sb, bk_sb, kwT, kts[4], 1, 0),
                             lambda: emit_qk(wk_sb, bk_sb, kwT, kts[5], 1, 1),
                             lambda: emit_qk(wk_sb, bk_sb, kwT, kts[6], 1, 2),
                             lambda: emit_qk(wk_sb, bk_sb, kwT, kts[7], 1, 3),
                             lambda: emit_qk(wq_sb, bq_sb, qwT, qts[4], 1, 0),
                             dma(qts, 5, qT, "qt5"),
                             dma(qts, 6, qT, "qt6")],
                    (1, 0): [lambda: emit_qk(wq_sb, bq_sb, qwT, qts[5], 1, 1),
                             dma(qts, 7, qT, "qt7")],
                    # gth0 loads at (1,1), both halves on gpsimd: it must be
                    # resident BEFORE A2A-1 starts (regular DMA queues
                    # starve to ~11GB/s while a collective is in flight, and
                    # a read of a collective output scheduled after A2A-1's
                    # trigger gets a wait on A2A-1 itself). gpsimd may block
                    # until A2A-0 completes; only stg h1 halves queue behind
                    # it, and those aren't needed until the A2A-1 trigger.
                    (1, 1): [lambda: emit_qk(wq_sb, bq_sb, qwT, qts[6], 1, 2),
                             lambda: emit_gth(gth0, cout0, nc.gpsimd,
                                              nc.gpsimd)],
                    (1, 2): [lambda: emit_qk(wq_sb, bq_sb, qwT, qts[7], 1, 3)],
                    (1, 3): [],
                }
                # the last slot of each batch interleaves both PV chains and
                # fires that batch's A2A immediately: batch 0's exchange
                # then hides under batch-1 attention, batch 1's tail
                # exposure starts as early as possible
                pair_slots = {(0, NCH - 1), (1, NCH - 1)}
                pend = None
                for b in range(2):
                    for ch in range(NCH):
                        prs = emit_scores(b, ch, range(KT // 2))
                        if pend is not None:
                            emit_pvs(*pend, dhs=(1,))
                            pend = None
                        for fn in aux_mid.get((b, ch), ()):
                            fn()
                        prs += emit_scores(b, ch, range(KT // 2, KT))
                        for fn in aux_pre.get((b, ch), ()):
                            fn()
                        if (b, ch) in pair_slots:
                            emit_pv_pair(b, ch, prs)
                            emit_a2a(b)
                        else:
                            emit_pvs(b, ch, prs, dhs=(0,))
                            pend = (b, ch, prs)
                        for fn in aux[(b, ch)]:
                            fn()

                if taps:
                    nc.sync.dma_start(out=taps["tqwT"][:], in_=qwT[:])
                    nc.sync.dma_start(out=taps["tkwT"][:], in_=kwT[:])
                    nc.sync.dma_start(out=taps["tvwx"][:], in_=vwx[:])

                # ---- tail ----------------------------------------------
                # batch-0 out-projection runs here, AFTER the A2A-1 trigger:
                # it covers the collective's latency with real work (the
                # attention slots it vacated were tensor-bound, so the
                # trigger also fires earlier); warm matmuls bridge any
                # remaining wait so batch-1's projection starts at full
                # clock.
                emit_oproj(0, gth0, 0)
                emit_oproj(0, gth0, 1)
                warm = aps.tile([128, QW], F32, name="ps", tag="ps")
                for i in range(26):
                    nc.tensor.matmul(warm[:], onesb[:], bo_sb[:, 0:QW],
                                     start=(i == 0), stop=(i == 25))
                # gth1 loads post-collective at full rate; the scalar queue
                # is idle once the exp stream ends, so use all three queues
                src1 = cout1.rearrange("(k p) n -> p k n", p=128)
                nc.sync.dma_start(out=gth1[:, 0:3, :], in_=src1[:, 0:3, :])
                nc.scalar.dma_start(out=gth1[:, 3:6, :], in_=src1[:, 3:6, :])
                nc.gpsimd.dma_start(out=gth1[:, 6:8, :], in_=src1[:, 6:8, :])
                emit_oproj(1, gth1, 0)
                emit_oproj(1, gth1, 1)

    nc.compile()
    return nc


def _get_nc():
    global _CACHED_NC
    if _CACHED_NC is None:
        _CACHED_NC = _build()
    return _CACHED_NC


def kernel(q, k, v, Wq, bq, Wk, bk, Wv, bv, Wo, bo, _return_results=False):
    q, k, v = (np.asarray(x, np.float32) for x in (q, k, v))
    Wq, bq, Wk, bk, Wv, bv, Wo, bo = (
        np.asarray(x, np.float32) for x in (Wq, bq, Wk, bk, Wv, bv, Wo, bo))

    nc = _get_nc()

    def arrange(x):  # [B, S, D] -> [2*NCH, 128, DKT, QW], same for all cores
        per_b = [np.ascontiguousarray(
            x[b].T.reshape(DKT, 128, NCH, QW).transpose(2, 1, 0, 3))
            for b in range(B)]
        return np.concatenate(per_b, axis=0).astype(ml_dtypes.bfloat16)

    qA, kA, vA = arrange(q), arrange(k), arrange(v)

    # Wo rows permuted to gathered-channel order (same for all cores)
    perm = np.empty(D, np.int64)
    for j in range(8):
        for dh in range(2):
            for d0 in range(64):
                perm[j * 128 + dh * 64 + d0] = (2 * j + dh) * 64 + d0
    woA = np.ascontiguousarray(
        Wo[perm].reshape(DKT, 128, D).transpose(1, 0, 2)).astype(
        ml_dtypes.bfloat16)
    identA = np.eye(128, dtype=np.float32).astype(ml_dtypes.bfloat16)
    boA = bo.reshape(1, D).astype(ml_dtypes.bfloat16)

    def warrange(w):  # [D, n] -> [128, DKT, n]
        n = w.shape[1]
        return np.ascontiguousarray(
            w.reshape(DKT, 128, n).transpose(1, 0, 2)).astype(
            ml_dtypes.bfloat16)

    in_maps = []
    for c in range(N_CORES):
        cols = slice(c * DPC, (c + 1) * DPC)
        in_maps.append({
            "qT": qA, "kT": kA, "vT": vA,
            "wq": warrange(Wq[:, cols]),
            "wk": warrange(Wk[:, cols]),
            "wv": warrange(Wv[:, cols]),
            "bq1": np.ascontiguousarray(bq[cols].reshape(128, 1)),
            "bk1": np.ascontiguousarray(bk[cols].reshape(128, 1)),
            "bv1": np.ascontiguousarray(bv[cols].reshape(128, 1)),
            "wo1": woA, "bo1": boA, "ident": identA,
        })

    res = run_bass_kernel_spmd(nc, in_maps, core_ids=list(range(N_CORES)))

    full = np.empty((B, S, D), np.float32)
    for c in range(N_CORES):
        o = np.asarray(res.results[c]["out"], ml_dtypes.bfloat16)
        for b in range(B):
            full[b, c * QO:(c + 1) * QO] = o[b].astype(np.float32)
    if _return_results:
        return full, res
    return full


# revision 59
# speedup vs baseline: 1.0611x; 1.0350x over previous
"""Multi-head attention (B=2, S=2048, D=1024, H=16) on 8 Trainium2 cores.

Sharding: head x batch. Core c handles heads {2c, 2c+1} for BOTH batches
(instead of 4 heads x 1 batch). This makes the post-attention exchange a
clean 8-way AllToAll per batch with zero redundancy: core c sends its
[128ch, 512q] blocks and receives exactly its [1024ch, 256q] output slice
(core c owns queries [256c, 256c+256) of each batch). Compared to the
batch-split sharding this halves the A2A payload (2 x 512KB, batch-0's
exchange hidden under batch-1 attention) and halves the output projection
(contract 1024 real channels, no zero-padded half).

Per core:
  1. Projections, interleaved with attention so the PE never idles:
     qw^T/kw^T in transposed layout (bf16 -- full-rate scores matmuls and
     half-size LDWEIGHTS vs fp32r), bias fused via DVE per-partition add;
     vw first as vw^T (N=512 matmuls, per-partition DVE bias), then PE
     transposes [128,128] blocks into the natural [kpos, ch] layout the
     PV matmul needs, with static memset ones columns for the fused
     softmax-denominator sums.
  2. Attention per (batch, q-chunk): scores^T [k,q] with K=64 matmuls
     packed two-per-array via row strips (head A partitions 0-63, head B
     64-127); exp on ACT (scale=1/8, no max subtraction -- scores are
     N(0,1)); PV+sums in one matmul stream via [ones|vw] lhsT; normalize
     with reciprocal_approx_fast. ACT is saturated here; projections for
     the other batch and the first batch's output projection fill the
     tensor-engine slack.
  3. Two 8-way AllToAlls (one per batch). Batch 0's fires mid-kernel and
     hides under batch-1 attention; only batch 1's is tail-exposed.
  4. Output projection per batch: bias first (start=True ones-matmul,
     PSUM-resident during the A2A wait), then 8 contraction steps over
     the gathered [1024ch, 256q] slice.
Host assembles the 8 disjoint [2, 256, 1024] slices.
"""
import contextlib
import ctypes
import os
import sys
import types

import ml_dtypes
import numpy as np

for _p in ("/opt/trn_rl_repo", os.path.expanduser("~/.axon_site/_ro/trn_rl_repo")):
    if os.path.isdir(_p) and _p not in sys.path:
        sys.path.insert(0, _p)
        break


def _install_ntff_hook():
    """run_bass_kernel_spmd(trace=True) under axon imports antenv.axon_hooks,
    which this image lacks; provide it so tracing degrades gracefully."""
    if "antenv.axon_hooks" in sys.modules:
        return
    mod = types.ModuleType("antenv.axon_hooks")
    state = {"hook": None}
    mod.set_axon_ntff_profile_hook = lambda h: state.__setitem__("hook", h)
    mod.get_axon_ntff_profile_hook = lambda: state["hook"]
    sys.modules["antenv.axon_hooks"] = mod
    try:
        import antenv

        antenv.axon_hooks = mod
    except ImportError:
        pass
    so_path = "/opt/axon/libaxon_pjrt.so"
    try:
        lib = ctypes.CDLL(so_path)
        if not hasattr(lib, "axon_start_nrt_profile"):
            return
        lib.axon_start_nrt_profile.argtypes = [
            ctypes.POINTER(ctypes.c_int64), ctypes.c_size_t]
        lib.axon_start_nrt_profile.restype = ctypes.c_int64
        lib.axon_stop_nrt_profile.argtypes = [ctypes.c_char_p]
        lib.axon_stop_nrt_profile.restype = ctypes.c_int64

        @contextlib.contextmanager
        def _ctx(output_dir, device_ids):
            import jax

            jax.devices()
            if device_ids:
                ids = (ctypes.c_int64 * len(device_ids))(*device_ids)
                rc = lib.axon_start_nrt_profile(ids, len(device_ids))
            else:
                rc = lib.axon_start_nrt_profile(None, 0)
            if rc != 0:
                raise RuntimeError(f"axon_start_nrt_profile rc={rc}")
            try:
                yield
            finally:
                n = lib.axon_stop_nrt_profile(str(output_dir).encode())
                print(f"profile: {n} ntff file(s) in {output_dir}",
                      file=sys.stderr)

        state["hook"] = _ctx
    except OSError:
        pass


_install_ntff_hook()

import concourse.bacc as bacc  # noqa: E402
import concourse.mybir as mybir  # noqa: E402
import concourse.tile as tile  # noqa: E402
from concourse.bass_utils import run_bass_kernel_spmd  # noqa: E402

F32 = mybir.dt.float32
BF16 = mybir.dt.bfloat16
AF = mybir.ActivationFunctionType
MUL = mybir.AluOpType.mult
ADD = mybir.AluOpType.add

N_CORES = 8
B, S, D, H, HD = 2, 2048, 1024, 16, 64
DPC = 2 * HD       # 128 projection columns per core (2 heads)
NCH = 4            # q chunks of 512 per batch
QW = S // NCH      # 512
QO = 256           # output queries per (core, batch)
KT = S // 128      # 16 k-position tiles per batch
DKT = D // 128     # 8 d_model contraction tiles

_CACHED_NC = None


def _build():
    nc = bacc.Bacc("TRN2", target_bir_lowering=False, debug=False,
                   num_devices=N_CORES)

    # x tensors hold BOTH batches: chunk index cidx = b*4 + ch, arranged as
    # [cidx, partition(d_in%128), k-tile(d_in//128), seq] so chunk DMAs read
    # 8KB-contiguous runs per partition
    qT = nc.dram_tensor("qT", [2 * NCH, 128, DKT, QW], BF16,
                        kind="ExternalInput").ap()
    kT = nc.dram_tensor("kT", [2 * NCH, 128, DKT, QW], BF16,
                        kind="ExternalInput").ap()
    vT = nc.dram_tensor("vT", [2 * NCH, 128, DKT, QW], BF16,
                        kind="ExternalInput").ap()
    wq = nc.dram_tensor("wq", [128, DKT, DPC], BF16,
                        kind="ExternalInput").ap()
    wk = nc.dram_tensor("wk", [128, DKT, DPC], BF16,
                        kind="ExternalInput").ap()
    wv = nc.dram_tensor("wv", [128, DKT, DPC], BF16,
                        kind="ExternalInput").ap()
    bq1 = nc.dram_tensor("bq1", [128, 1], F32, kind="ExternalInput").ap()
    bk1 = nc.dram_tensor("bk1", [128, 1], F32, kind="ExternalInput").ap()
    bv1 = nc.dram_tensor("bv1", [128, 1], F32, kind="ExternalInput").ap()
    # Wo rows permuted to the gathered-channel order: row (j*128 + dh*64 + d)
    # holds Wo[(2j+dh)*64 + d, :]
    wo1 = nc.dram_tensor("wo1", [128, DKT, D], BF16,
                         kind="ExternalInput").ap()
    bo1 = nc.dram_tensor("bo1", [1, D], BF16, kind="ExternalInput").ap()
    ident = nc.dram_tensor("ident", [128, 128], BF16,
                           kind="ExternalInput").ap()
    out = nc.dram_tensor("out", [2, QO, D], BF16, kind="ExternalOutput").ap()

    taps = {}
    if os.environ.get("DEBUG_TAPS"):
        taps["tqwT"] = nc.dram_tensor("tqwT", [128, 2, S], BF16,
                                      kind="ExternalOutput").ap()
        taps["tkwT"] = nc.dram_tensor("tkwT", [128, 2, S], BF16,
                                      kind="ExternalOutput").ap()
        taps["tvwx"] = nc.dram_tensor("tvwx", [128, 2, KT, 256], BF16,
                                      kind="ExternalOutput").ap()

    with tile.TileContext(nc) as tc:
        with tc.tile_pool(name="xw", bufs=1) as xw, \
             tc.tile_pool(name="dram", bufs=1, space="DRAM") as dram:
            # long-lived tiles
            qwT = xw.tile([128, 2, S], BF16, name="qwT")   # [dh*64+d, b, q]
            kwT = xw.tile([128, 2, S], BF16, name="kwT")
            # [kpos%128, b, kt, (ones64|vw64) x2 dh]
            vwx = xw.tile([128, 2, KT, 256], BF16, name="vwx")
            bq_sb = xw.tile([128, 1], F32, name="bq_sb")
            bk_sb = xw.tile([128, 1], F32, name="bk_sb")
            bv_sb = xw.tile([128, 1], F32, name="bv_sb")
            onesb = xw.tile([1, 128], BF16, name="onesb")
            bo_sb = xw.tile([1, D], BF16, name="bo_sb")
            id_sb = xw.tile([128, 128], BF16, name="id_sb")
            wo_sb = xw.tile([128, DKT, D], BF16, name="wo_sb")
            gth0 = xw.tile([128, DKT, QO], BF16, name="gth0")
            gth1 = xw.tile([128, DKT, QO], BF16, name="gth1")
            wq_sb = xw.tile([128, DKT, DPC], BF16, name="wq_sb")
            wk_sb = xw.tile([128, DKT, DPC], BF16, name="wk_sb")
            wv_sb = xw.tile([128, DKT, DPC], BF16, name="wv_sb")

            ones_f = xw.tile([1, 128], F32, name="ones_f")
            nc.gpsimd.memset(ones_f[:], 1.0)
            nc.vector.tensor_copy(onesb[:], ones_f[:])
            # static ones columns of vwx (softmax-denominator lhsT rows);
            # two 4D memsets (one per dh) keep the APs within dim limits
            nc.vector.memset(vwx[:, :, :, 0:64], 1.0)
            nc.vector.memset(vwx[:, :, :, 128:192], 1.0)

            # A2A staging: cin rows [(2ch+h)*128 + dh*64 + d] = ctx^T rows,
            # chunk j of 128 rows goes to core j (= q block [256j, 256j+256))
            cin0 = dram.tile([1024, QO], BF16, name="cin0")
            cout0 = dram.tile([1024, QO], BF16, name="cout0")
            cin1 = dram.tile([1024, QO], BF16, name="cin1")
            cout1 = dram.tile([1024, QO], BF16, name="cout1")
            cins, couts = (cin0, cin1), (cout0, cout1)
            # full-size scratch warmup exchange absorbs the NRT
            # first-collective setup cost off the critical path. The payload
            # MUST match the real A2As: a smaller warmup leaves the real
            # 512KB exchanges running 3-5x slower (NRT sizes channel state
            # from the first op), which also starves concurrent input DMA.
            ccw_in = dram.tile([1024, QO], BF16, name="ccw_in")
            ccw_out = dram.tile([1024, QO], BF16, name="ccw_out")

            # ---- startup DMAs, strict need-order -------------------------
            # first matmul needs kt chunk 0 + wk only: keep them unblocked.
            # Each hardware DMA queue sustains only ~115GB/s, so chunk halves
            # rotate across all three DMA-capable engines (sync/scalar/
            # gpsimd) to keep aggregate input bandwidth near the HBM limit.
            _xq = [nc.sync, nc.scalar, nc.gpsimd]
            _xqi = [0]

            def xchunk_dma(xtp, x_dram, cidx, name, qa=None, qb=None):
                t = xtp.tile([128, DKT, QW], BF16, name=name, tag="xt")
                h = DKT // 2
                if qa is None:
                    qa = _xq[_xqi[0] % 3]
                    qb = _xq[(_xqi[0] + 1) % 3]
                    _xqi[0] += 2
                qa.dma_start(out=t[:, 0:h, :], in_=x_dram[cidx][:, 0:h, :])
                qb.dma_start(out=t[:, h:DKT, :], in_=x_dram[cidx][:, h:DKT, :])
                return t

            with tc.tile_pool(name="xt", bufs=5) as xtp, \
                 tc.tile_pool(name="vts", bufs=2) as vts, \
                 tc.tile_pool(name="stg", bufs=8) as stp, \
                 tc.tile_pool(name="osb", bufs=2) as osb, \
                 tc.tile_pool(name="prp", bufs=36) as prp, \
                 tc.tile_pool(name="sps", bufs=2, space="PSUM") as sps, \
                 tc.tile_pool(name="vps", bufs=2, space="PSUM") as vps, \
                 tc.tile_pool(name="aps", bufs=2, space="PSUM") as aps:

                # x-chunk DMA emission must match consumption order exactly:
                # the 4-buffer ring makes chunk i's DMA wait on chunk i-4's
                # last reader, so out-of-order emission would deadlock the
                # in-order engines.
                def xchunk3(x_dram, cidx, name):
                    """Chunk split three ways across all DMA queues, in
                    strict consumption order -- every queue then delivers
                    chunks in the same order the PE consumes them."""
                    t = xtp.tile([128, DKT, QW], BF16, name=name, tag="xt")
                    nc.sync.dma_start(out=t[:, 0:3, :],
                                      in_=x_dram[cidx][:, 0:3, :])
                    nc.scalar.dma_start(out=t[:, 3:6, :],
                                        in_=x_dram[cidx][:, 3:6, :])
                    nc.gpsimd.dma_start(out=t[:, 6:8, :],
                                        in_=x_dram[cidx][:, 6:8, :])
                    return t

                kts, qts, vtss = {}, {}, {}
                nc.gpsimd.dma_start(out=wk_sb[:], in_=wk[:])
                kts[0] = xchunk3(kT, 0, "kt0")
                nc.gpsimd.dma_start(out=bk_sb[:], in_=bk1[:])
                nc.gpsimd.dma_start(out=bq_sb[:], in_=bq1[:])
                nc.gpsimd.dma_start(out=bv_sb[:], in_=bv1[:])
                nc.gpsimd.dma_start(out=bo_sb[:], in_=bo1[:])
                nc.gpsimd.dma_start(out=id_sb[:], in_=ident[:])
                kts[1] = xchunk3(kT, 1, "kt1")
                for c in range(2, 4):
                    kts[c] = xchunk_dma(xtp, kT, c, f"kt{c}")
                nc.gpsimd.dma_start(out=wq_sb[:], in_=wq[:])
                qts[0] = xchunk_dma(xtp, qT, 0, "qt0")
                # v chunks after q0: the first scores only need k+q data,
                # vw consumption starts a slot later
                nc.gpsimd.dma_start(out=wv_sb[:], in_=wv[:])
                for c in range(4):
                    vtss[c] = xchunk_dma(xtp, vT, c, f"vt{c}")
                # collective warmup fires now; CC setup cost retires long
                # before the first real A2A
                nc.gpsimd.collective_compute(
                    "AllToAll", mybir.AluOpType.bypass,
                    replica_groups=[list(range(N_CORES))],
                    ins=[ccw_in[:].opt()], outs=[ccw_out[:].opt()])
                qts[1] = xchunk_dma(xtp, qT, 1, "qt1")
                nc.gpsimd.dma_start(out=wo_sb[:], in_=wo1[:])
                # batch-1 chunk DMAs are emitted just-in-time inside the
                # attention loop: a dma_start blocks its issuing engine
                # until the tile pool slot frees, so emitting them all here
                # would stall the sync/gpsimd queues (delaying the stg
                # writes that feed the A2As)

                # in-loop chunk DMAs must avoid the scalar queue: a blocked
                # dma_start would stall the ACT engine mid-exp-stream
                _jq = [0]

                def dma(store, key, x_dram, name):
                    def fn():
                        qa, qb = ((nc.sync, nc.gpsimd) if _jq[0] % 2 == 0
                                  else (nc.gpsimd, nc.sync))
                        _jq[0] += 1
                        store[key] = xchunk_dma(xtp, x_dram, key, name,
                                                qa, qb)
                    return fn

                # ---- projection emitters --------------------------------
                def emit_qk(w_sb, b_sb, dstT, xt, b, ch):
                    """One chunk of qw^T/kw^T: [128 dout, 512 q] += bias."""
                    ps = aps.tile([128, QW], F32, name="ps", tag="ps")
                    for kk in range(DKT):
                        nc.tensor.matmul(ps[:], w_sb[:, kk, :], xt[:, kk, :],
                                         start=(kk == 0), stop=(kk == DKT - 1))
                    nc.vector.tensor_scalar_add(
                        dstT[:, b, ch * QW:(ch + 1) * QW], ps[:],
                        b_sb[:, 0:1])

                def emit_vw(xt, b, ch):
                    """One chunk of vw: project transposed (N=512), add bias
                    per-partition, then PE-transpose 128x128 blocks into the
                    natural [kpos, ch] slots of vwx."""
                    ps = aps.tile([128, QW], F32, name="ps", tag="ps")
                    for kk in range(DKT):
                        nc.tensor.matmul(ps[:], wv_sb[:, kk, :], xt[:, kk, :],
                                         start=(kk == 0), stop=(kk == DKT - 1))
                    vt_sb = vts.tile([128, QW], BF16, name="vt_sb", tag="vt")
                    nc.vector.tensor_scalar_add(vt_sb[:], ps[:], bv_sb[:, 0:1])
                    for s in range(4):
                        kt = ch * 4 + s
                        tp = vps.tile([128, QW], F32, name="pv", tag="pv")
                        tpb = tp[:].bitcast(BF16)[:, 0:128]
                        nc.tensor.transpose(
                            tpb, vt_sb[:, s * 128:(s + 1) * 128], id_sb[:])
                        dst = vwx[:, b, kt, :].rearrange(
                            "p (d c) -> p d c", d=2)
                        nc.vector.tensor_copy(
                            dst[:, :, 64:128],
                            tpb.rearrange("p (d c) -> p d c", d=2))

                # ---- attention emitters ---------------------------------
                def emit_scores(b, ch, kt_range):
                    prs = []
                    for kt in kt_range:
                        sq = sps.tile([128, 2, QW], F32, name="sq", tag="sq")
                        for dh in range(2):
                            nc.tensor.matmul(
                                sq[:, dh, :],
                                kwT[dh * 64:(dh + 1) * 64, b,
                                    kt * 128:(kt + 1) * 128],
                                qwT[dh * 64:(dh + 1) * 64, b,
                                    ch * QW:(ch + 1) * QW],
                                start=True, stop=True)
                        pr = prp.tile([128, 2, QW], BF16, name="pr", tag="pr")
                        nc.scalar.activation(pr[:], sq[:], AF.Exp, scale=0.125)
                        prs.append(pr)
                    return prs

                def emit_pvs(b, ch, prs, dhs=(0, 1)):
                    for dh in dhs:
                        pv = vps.tile([128, QW], F32, name="pv", tag="pv")
                        for kt in range(KT):
                            nc.tensor.matmul(
                                pv[:],
                                vwx[:, b, kt, dh * 128:(dh + 1) * 128],
                                prs[kt][:, dh, :],
                                start=(kt == 0), stop=(kt == KT - 1))
                        # sums land at PSUM rows 0:64 (ones first in lhsT)
                        rec = stp.tile([64, QW], F32, name="rec", tag="rec")
                        nc.vector.reciprocal_approx_fast(rec[:], pv[0:64, :])
                        stg = stp.tile([64, QW], BF16, name="stg", tag="stg")
                        nc.vector.tensor_tensor(stg[:], pv[64:128, :], rec[:],
                                                MUL)
                        # batch-1 slots keep these off gpsimd: gth0's
                        # dma_start blocks that queue until A2A-0 completes,
                        # and cin writes stuck behind it stall the stg ring
                        # (and with it DVE -> PSUM -> PE) when the
                        # collective runs long
                        qpair = (((0, nc.sync), (1, nc.gpsimd)) if b == 0
                                 else ((0, nc.sync), (1, nc.sync)))
                        for h, qeng in qpair:
                            r0 = (2 * ch + h) * 128 + dh * 64
                            qeng.dma_start(
                                out=cins[b][r0:r0 + 64, :],
                                in_=stg[:, h * QO:(h + 1) * QO])

                def emit_pv_pair(b, ch, prs):
                    """Both heads' PV chains interleaved per k-tile: after
                    the slot's last exp, only ~2 matmuls remain instead of a
                    full 16-matmul chain. Used for the final slot so the
                    tail A2A triggers as early as possible."""
                    pvs = [vps.tile([128, QW], F32, name="pv", tag="pv")
                           for _ in range(2)]
                    for kt in range(KT):
                        for dh in range(2):
                            nc.tensor.matmul(
                                pvs[dh][:],
                                vwx[:, b, kt, dh * 128:(dh + 1) * 128],
                                prs[kt][:, dh, :],
                                start=(kt == 0), stop=(kt == KT - 1))
                    for dh in range(2):
                        pv = pvs[dh]
                        rec = stp.tile([64, QW], F32, name="rec", tag="rec")
                        nc.vector.reciprocal_approx_fast(rec[:], pv[0:64, :])
                        stg = stp.tile([64, QW], BF16, name="stg", tag="stg")
                        nc.vector.tensor_tensor(stg[:], pv[64:128, :], rec[:],
                                                MUL)
                        for h, qeng in ((0, nc.sync), (1, nc.gpsimd)):
                            r0 = (2 * ch + h) * 128 + dh * 64
                            qeng.dma_start(
                                out=cins[b][r0:r0 + 64, :],
                                in_=stg[:, h * QO:(h + 1) * QO])

                def emit_a2a(b):
                    nc.gpsimd.collective_compute(
                        "AllToAll", mybir.AluOpType.bypass,
                        replica_groups=[list(range(N_CORES))],
                        ins=[cins[b][:].opt()],
                        outs=[couts[b][:].opt()])

                def emit_gth(gth, cout, qa, qb):
                    src = cout.rearrange("(k p) n -> p k n", p=128)
                    qa.dma_start(out=gth[:, 0:DKT // 2, :],
                                 in_=src[:, 0:DKT // 2, :])
                    qb.dma_start(out=gth[:, DKT // 2:DKT, :],
                                 in_=src[:, DKT // 2:DKT, :])

                def emit_oproj(b, gth, mb, qs=None):
                    """Output projection for q-block mb of batch b; each
                    512-column half stores out as soon as its copy lands."""
                    if qs is None:
                        qs = (nc.sync, nc.sync)
                    for nch in range(2):
                        ps = aps.tile([128, QW], F32, name="ps", tag="ps")
                        nc.tensor.matmul(ps[:], onesb[:],
                                         bo_sb[:, nch * QW:(nch + 1) * QW],
                                         start=True, stop=False)
                        for kk in range(DKT):
                            nc.tensor.matmul(
                                ps[:], gth[:, kk, mb * 128:(mb + 1) * 128],
                                wo_sb[:, kk, nch * QW:(nch + 1) * QW],
                                start=False, stop=(kk == DKT - 1))
                        osb_t = osb.tile([128, QW], BF16, name="osb_t",
                                         tag="osb")
                        nc.vector.tensor_copy(osb_t[:], ps[:])
                        qs[nch].dma_start(
                            out=out[b, mb * 128:(mb + 1) * 128,
                                    nch * QW:(nch + 1) * QW],
                            in_=osb_t[:])

                # ---- phase 1a: batch-0 k/q projections ------------------
                # only what the first scores read; vw-b0 slides into slot
                # (0, 0) between the score groups (before that slot's PV,
                # which reads all of vwx batch 0 -- reads must follow
                # writes in trace order or the in-order engines deadlock).
                for ch in range(4):
                    emit_qk(wk_sb, bk_sb, kwT, kts[ch], 0, ch)
                emit_qk(wq_sb, bq_sb, qwT, qts[0], 0, 0)

                # ---- attention with aux work threaded through -----------
                # aux_mid[(b, ch)] is emitted between the slot's two score
                # groups, aux[(b, ch)] after its dh0 PV; everything a later
                # slot's scores/PVs read is emitted ahead of its first use.
                aux_mid = {
                    (0, 0): [lambda: emit_vw(vtss[0], 0, 0),
                             lambda: emit_vw(vtss[1], 0, 1)],
                }
                aux_pre = {
                    (0, 0): [lambda: emit_vw(vtss[2], 0, 2),
                             lambda: emit_vw(vtss[3], 0, 3)],
                }
                aux = {
                    (0, 0): [lambda: emit_qk(wq_sb, bq_sb, qwT, qts[1], 0, 1),
                             dma(qts, 2, qT, "qt2"),
                             dma(vtss, 4, vT, "vt4")],
                    (0, 1): [lambda: emit_qk(wq_sb, bq_sb, qwT, qts[2], 0, 2),
                             lambda: emit_vw(vtss[4], 1, 0),
                             dma(qts, 3, qT, "qt3"),
                             dma(vtss, 5, vT, "vt5"),
                             dma(vtss, 6, vT, "vt6")],
                    (0, 2): [lambda: emit_qk(wq_sb, bq_sb, qwT, qts[3], 0, 3),
                             lambda: emit_vw(vtss[5], 1, 1),
                             lambda: emit_vw(vtss[6], 1, 2),
                             dma(vtss, 7, vT, "vt7"),
                             dma(kts, 4, kT, "kt4"),
                             dma(kts, 5, kT, "kt5"),
                             dma(kts, 6, kT, "kt6"),
                             dma(kts, 7, kT, "kt7")],
                    (0, 3): [lambda: emit_vw(vtss[7], 1, 3),
                             dma(qts, 4, qT, "qt4"),
                             lambda: emit_qk(wk_sb, bk_sb, kwT, kts[4], 1, 0),
                             lambda: emit_qk(wk_sb, bk_sb, kwT, kts[5], 1, 1),
                             lambda: emit_qk(wk_sb, bk_sb, kwT, kts[6], 1, 2),
                             lambda: emit_qk(wk_sb, bk_sb, kwT, kts[7], 1, 3),
                             lambda: emit_qk(wq_sb, bq_sb, qwT, qts[4], 1, 0),
                             dma(qts, 5, qT, "qt5"),
                             dma(qts, 6, qT, "qt6")],
                    (1, 0): [lambda: emit_qk(wq_sb, bq_sb, qwT, qts[5], 1, 1),
                             dma(qts, 7, qT, "qt7")],
                    # gth0 loads at (1,1), both halves on gpsimd: it must be
                    # resident BEFORE A2A-1 starts (regular DMA queues
                    # starve to ~11GB/s while a collective is in flight, and
                    # a read of a collective output scheduled after A2A-1's
                    # trigger gets a wait on A2A-1 itself). gpsimd may block
                    # until A2A-0 completes; only stg h1 halves queue behind
                    # it, and those aren't needed until the A2A-1 trigger.
                    (1, 1): [lambda: emit_qk(wq_sb, bq_sb, qwT, qts[6], 1, 2),
                             lambda: emit_gth(gth0, cout0, nc.gpsimd,
                                              nc.gpsimd)],
                    (1, 2): [lambda: emit_qk(wq_sb, bq_sb, qwT, qts[7], 1, 3)],
                    (1, 3): [],
                }
                # the last slot of each batch interleaves both PV chains and
                # fires that batch's A2A immediately: batch 0's exchange
                # then hides under batch-1 attention, batch 1's tail
                # exposure starts as early as possible
                pair_slots = {(0, NCH - 1), (1, NCH - 1)}
                pend = None
                for b in range(2):
                    for ch in range(NCH):
                        prs = emit_scores(b, ch, range(KT // 2))
                        if pend is not None:
                            emit_pvs(*pend, dhs=(1,))
                            pend = None
                        for fn in aux_mid.get((b, ch), ()):
                            fn()
                        prs += emit_scores(b, ch, range(KT // 2, KT))
                        for fn in aux_pre.get((b, ch), ()):
                            fn()
                        if (b, ch) in pair_slots:
                            emit_pv_pair(b, ch, prs)
                            emit_a2a(b)
                        else:
                            emit_pvs(b, ch, prs, dhs=(0,))
                            pend = (b, ch, prs)
                        for fn in aux[(b, ch)]:
                            fn()

                if taps:
                    nc.sync.dma_start(out=taps["tqwT"][:], in_=qwT[:])
                    nc.sync.dma_start(out=taps["tkwT"][:], in_=kwT[:])
                    nc.sync.dma_start(out=taps["tvwx"][:], in_=vwx[:])

                # ---- tail ----------------------------------------------
                # batch-0 out-projection runs here, AFTER the A2A-1 trigger:
                # it covers the collective's latency with real work (the
                # attention slots it vacated were tensor-bound, so the
                # trigger also fires earlier); warm matmuls bridge any
                # remaining wait so batch-1's projection starts at full
                # clock.
                emit_oproj(0, gth0, 0)
                emit_oproj(0, gth0, 1)
                warm = aps.tile([128, QW], F32, name="ps", tag="ps")
                # K=1 warm matmuls stream at ~200ns each: 64 of them
                # bridge ~13us of the post-trigger collective window, so
                # the batch-1 projection starts at operating p-state
                for i in range(64):
                    nc.tensor.matmul(warm[:], onesb[:], bo_sb[:, 0:QW],
                                     start=(i == 0), stop=(i == 63))
                # gth1 loads post-collective at full rate; the scalar queue
                # is idle once the exp stream ends, so use all three queues
                src1 = cout1.rearrange("(k p) n -> p k n", p=128)
                nc.sync.dma_start(out=gth1[:, 0:3, :], in_=src1[:, 0:3, :])
                nc.scalar.dma_start(out=gth1[:, 3:6, :], in_=src1[:, 3:6, :])
                nc.gpsimd.dma_start(out=gth1[:, 6:8, :], in_=src1[:, 6:8, :])
                emit_oproj(1, gth1, 0)
                emit_oproj(1, gth1, 1)

    nc.compile()
    return nc


def _get_nc():
    global _CACHED_NC
    if _CACHED_NC is None:
        _CACHED_NC = _build()
    return _CACHED_NC


def kernel(q, k, v, Wq, bq, Wk, bk, Wv, bv, Wo, bo, _return_results=False):
    q, k, v = (np.asarray(x, np.float32) for x in (q, k, v))
    Wq, bq, Wk, bk, Wv, bv, Wo, bo = (
        np.asarray(x, np.float32) for x in (Wq, bq, Wk, bk, Wv, bv, Wo, bo))

    nc = _get_nc()

    def arrange(x):  # [B, S, D] -> [2*NCH, 128, DKT, QW], same for all cores
        per_b = [np.ascontiguousarray(
            x[b].T.reshape(DKT, 128, NCH, QW).transpose(2, 1, 0, 3))
            for b in range(B)]
        return np.concatenate(per_b, axis=0).astype(ml_dtypes.bfloat16)

    qA, kA, vA = arrange(q), arrange(k), arrange(v)

    # Wo rows permuted to gathered-channel order (same for all cores)
    perm = np.empty(D, np.int64)
    for j in range(8):
        for dh in range(2):
            for d0 in range(64):
                perm[j * 128 + dh * 64 + d0] = (2 * j + dh) * 64 + d0
    woA = np.ascontiguousarray(
        Wo[perm].reshape(DKT, 128, D).transpose(1, 0, 2)).astype(
        ml_dtypes.bfloat16)
    identA = np.eye(128, dtype=np.float32).astype(ml_dtypes.bfloat16)
    boA = bo.reshape(1, D).astype(ml_dtypes.bfloat16)

    def warrange(w):  # [D, n] -> [128, DKT, n]
        n = w.shape[1]
        return np.ascontiguousarray(
            w.reshape(DKT, 128, n).transpose(1, 0, 2)).astype(
            ml_dtypes.bfloat16)

    in_maps = []
    for c in range(N_CORES):
        cols = slice(c * DPC, (c + 1) * DPC)
        in_maps.append({
            "qT": qA, "kT": kA, "vT": vA,
            "wq": warrange(Wq[:, cols]),
            "wk": warrange(Wk[:, cols]),
            "wv": warrange(Wv[:, cols]),
            "bq1": np.ascontiguousarray(bq[cols].reshape(128, 1)),
            "bk1": np.ascontiguousarray(bk[cols].reshape(128, 1)),
            "bv1": np.ascontiguousarray(bv[cols].reshape(128, 1)),
            "wo1": woA, "bo1": boA, "ident": identA,
        })

    res = run_bass_kernel_spmd(nc, in_maps, core_ids=list(range(N_CORES)))

    full = np.empty((B, S, D), np.float32)
    for c in range(N_CORES):
        o = np.asarray(res.results[c]["out"], ml_dtypes.bfloat16)
        for b in range(B):
            full[b, c * QO:(c + 1) * QO] = o[b].astype(np.float32)
    if _return_results:
        return full, res
    return full
